# revision 12
# baseline (speedup 1.0000x reference)
"""BiLSTM(2-layer, bidir) + 2x GraphConv + combine, on 8 TRN2 NeuronCores.

Sharding: branch-split data parallel. Cores 0-3 run branch 1 (x1/lstm1/w11/w21)
on graph quarters 0-3; cores 4-7 run branch 2 on the same quarters. Each core:
  - 2-layer bidirectional LSTM over its B graphs (T=35, D=300, H=256),
    feature-major layout, float32r matmuls, gates PSUM-accumulated.
  - GCN: per-3-graph-block dense normalized adjacency (edge-count matrices
    packed on host from the int32 edge lists), aggregation/projection as
    dense matmuls; rsqrt degree norms computed on device. The BiLSTM output
    is staged to DRAM in bf16 (t,b)-order and re-read SBUF-resident; the
    (t,b)->block-slot reorder happens inside the matmul lhsT access pattern.
  - mean-over-nodes -> rep^T; AllGather over {q, q+4} pairs; final
    tanh(rep1 - rep2) @ pred_w^T + pred_b -> [B, 2].
Host assembles [2048, 2] from cores 0-3.
"""
import sys
sys.path.insert(0, '/opt/trn_rl_repo')

import numpy as np
import ml_dtypes

import concourse.bass as bass
import concourse.bacc as bacc
import concourse.mybir as mybir
from concourse import tile
from concourse.bass_utils import run_bass_kernel_spmd

F32 = mybir.dt.float32
F32R = mybir.dt.float32r
BF16 = mybir.dt.bfloat16
AF = mybir.ActivationFunctionType
ALU = mybir.AluOpType

D = 300         # input size
H = 256         # lstm hidden per dir
G = 256         # gcn hidden
OUT = 2
T_FULL = 35
BG_FULL = 2048
EPER = 140
NCORES = 8

# permutation of the 4H gate dim: pytorch [i,f,g,o] -> [i,f,o,g]
GPERM = np.r_[0:2 * H, 3 * H:4 * H, 2 * H:3 * H]


def ceil_div(a, b):
    return (a + b - 1) // b


def build_program(NB, T):
    """One SPMD program for all 8 cores. NB graphs/core, T nodes/graph."""
    NBLK = ceil_div(NB, 3)
    NS = 3 * T                 # slots per block (<= 128)
    NB3 = NBLK * 3             # NB padded to multiple of 3
    REM = NB - 3 * (NBLK - 1)  # graphs in last block (1..3)

    nc = bacc.Bacc("TRN2", target_bir_lowering=False, debug=False,
                   num_devices=NCORES)

    def din(name, shape, dt=F32R):
        return nc.dram_tensor(name, shape, dt, kind="ExternalInput").ap()

    # ---- inputs ----
    xT = din("xT", [3, 128, T, NB])               # row 300 = ones, rest pad 0
    w0i = [din(f"w0i_d{d_}", [3, 128, 8, 128]) for d_ in range(2)]
    w0h = [din(f"w0h_d{d_}", [2, 128, 8, 128]) for d_ in range(2)]
    w1i = [din(f"w1i_d{d_}", [4, 128, 8, 128]) for d_ in range(2)]
    w1h = [din(f"w1h_d{d_}", [2, 128, 8, 128]) for d_ in range(2)]
    b1 = [din(f"b1_d{d_}", [128, 8], F32) for d_ in range(2)]
    cmat = din("cmat", [NBLK, 128, 128], BF16)    # per-block edge counts [s,d]
    w1g = din("w1g", [4, 128, G], BF16)           # gcn layer1 W [2H,G] (bf16)
    w2g = din("w2g", [2, 128, G])                 # gcn layer2 W [G,G] (f32r)
    linT = din("linT", [4, 128, G], BF16)         # lin_w^T [2H,G] (bf16)
    linb = din("linb", [1, G], BF16)              # lin_b row
    predT = din("predT", [2, 128, OUT])           # pred_w^T [G,2]
    predb = din("predb", [128, OUT], F32)         # pred_b replicated rows
    out_d = nc.dram_tensor("out", [NB, OUT], F32, kind="ExternalOutput").ap()

    # ---- consts (embedded in NEFF) ----
    ident = nc.inline_tensor(np.eye(128, dtype=np.float32), "ident").ap()
    ones_rowbf = nc.inline_tensor(np.ones((1, 128), dtype=ml_dtypes.bfloat16),
                                  "ones_rowbf").ap()
    onesb_np = np.zeros((128, 3), dtype=np.float32)
    for j in range(NS):
        onesb_np[j, j // T] = 1.0 / T
    ones_bcol = nc.inline_tensor(onesb_np, "ones_bcol").ap()
    ones_colbf = nc.inline_tensor(np.ones((128, 1), dtype=ml_dtypes.bfloat16),
                                  "ones_colbf").ap()

    # ---- scratch DRAM ----
    wfl0 = nc.dram_tensor("wfl0", [2, 2, 128, T, NB], F32R).ap()   # (dir,j,p,t,b)
    wfn = nc.dram_tensor("wfn", [2, 2, 128, T, NB], BF16).ap()     # L1 out, bf16
    m_dram = nc.dram_tensor("m_dram", [NBLK, 128, NS], F32R).ap()
    cc_in = nc.dram_tensor("cc_in", [G, NB], F32).ap()
    cc_out = nc.dram_tensor("cc_out", [2, G, NB], F32).ap()

    with tile.TileContext(nc) as tc:
        # persistent LSTM state
        h_sb = [nc.alloc_sbuf_tensor(f"h_d{d_}", [128, 2, NB], F32R).ap()
                for d_ in range(2)]
        c_sb = [nc.alloc_sbuf_tensor(f"c_d{d_}", [128, 2, NB], F32).ap()
                for d_ in range(2)]
        rI06g = nc.alloc_sbuf_tensor("rI06g", [128, NBLK], F32).ap()

        with tc.tile_pool(name="wpool", bufs=1) as wp, \
             tc.tile_pool(name="xpool", bufs=2) as xp, \
             tc.tile_pool(name="gate", bufs=2) as gp, \
             tc.tile_pool(name="gate1", bufs=1) as gp1, \
             tc.tile_pool(name="ps", bufs=2, space="PSUM") as ps:

            # ============ LSTM ============
            for layer in range(2):
                if layer == 0:
                    nkt = 3
                    kpart = [128, 128, D - 256 + 1]  # last tile: 44 chans + bias
                    wi_d, wh_d = w0i, w0h
                else:
                    nkt = 4
                    kpart = [128, 128, 128, 128]
                    wi_d, wh_d = w1i, w1h
                wi_sb, wh_sb, b1_sb = [], [], []
                for d_ in range(2):
                    wi = wp.tile([128, nkt, 8, 128], F32R, tag=f"wi{d_}")
                    nc.sync.dma_start(out=wi[:], in_=wi_d[d_].transpose([1, 0, 2, 3]))
                    wi_sb.append(wi)
                    wh = wp.tile([128, 2, 8, 128], F32R, tag=f"wh{d_}")
                    nc.sync.dma_start(out=wh[:], in_=wh_d[d_].transpose([1, 0, 2, 3]))
                    wh_sb.append(wh)
                    if layer == 1:
                        bt = wp.tile([128, 8], F32, tag=f"b1{d_}")
                        nc.sync.dma_start(out=bt[:], in_=b1[d_])
                        b1_sb.append(bt)
                    nc.gpsimd.memset(c_sb[d_][:], 0.0)

                for tau in range(T):
                    for d_ in range(2):
                        t = tau if d_ == 0 else T - 1 - tau
                        if layer == 0:
                            xt = xp.tile([128, 3, NB], F32R, tag=f"xt{d_}")
                            nc.sync.dma_start(out=xt[:], in_=xT[:, :, t, :].transpose([1, 0, 2]))
                            xkt = [xt[:kpart[k], k, :] for k in range(nkt)]
                        else:
                            xt = xp.tile([128, 2, 2, NB], F32R, tag=f"xt{d_}")
                            nc.sync.dma_start(
                                out=xt[:], in_=wfl0[:, :, :, t, :].transpose([2, 0, 1, 3]))
                            xkt = [xt[:, k // 2, k % 2, :] for k in range(nkt)]
                        chunks = []
                        for ch in range(2):
                            gps = ps.tile([128, 4, NB], F32, tag="gates")
                            for m in range(4):
                                mt = ch * 4 + m
                                for k in range(nkt):
                                    nc.tensor.matmul(
                                        gps[:, m, :], wi_sb[d_][:kpart[k], k, mt, :],
                                        xkt[k], start=(k == 0),
                                        stop=(tau == 0 and k == nkt - 1))
                                if tau > 0:
                                    for k in range(2):
                                        nc.tensor.matmul(
                                            gps[:, m, :], wh_sb[d_][:, k, mt, :],
                                            h_sb[d_][:, k, :], start=False,
                                            stop=(k == 1))
                            chunks.append(gps)
                        sigA = gp.tile([128, 4, NB], F32, tag=f"sigA{d_}")
                        sigB = gp.tile([128, 2, NB], F32, tag=f"sigB{d_}")
                        tg = gp.tile([128, 2, NB], F32, tag=f"tg{d_}")
                        if layer == 0:
                            nc.scalar.activation(sigA[:], chunks[0][:], AF.Sigmoid)
                            nc.scalar.activation(sigB[:], chunks[1][:, 0:2, :], AF.Sigmoid)
                            nc.scalar.activation(tg[:], chunks[1][:, 2:4, :], AF.Tanh)
                        else:
                            for m in range(4):
                                nc.scalar.activation(sigA[:, m, :], chunks[0][:, m, :],
                                                     AF.Sigmoid, bias=b1_sb[d_][:, m:m + 1])
                            for m in range(2):
                                nc.scalar.activation(sigB[:, m, :], chunks[1][:, m, :],
                                                     AF.Sigmoid, bias=b1_sb[d_][:, 4 + m:5 + m])
                                nc.scalar.activation(tg[:, m, :], chunks[1][:, 2 + m, :],
                                                     AF.Tanh, bias=b1_sb[d_][:, 6 + m:7 + m])
                        ig = gp1.tile([128, 2, NB], F32, tag=f"ig{d_}")
                        nc.gpsimd.tensor_tensor(ig[:], sigA[:, 0:2, :], tg[:], ALU.mult)
                        nc.vector.tensor_tensor(c_sb[d_][:], c_sb[d_][:],
                                                sigA[:, 2:4, :], ALU.mult)
                        nc.vector.tensor_tensor(c_sb[d_][:], c_sb[d_][:], ig[:], ALU.add)
                        tc_t = gp1.tile([128, 2, NB], F32, tag=f"tc{d_}")
                        nc.scalar.activation(tc_t[:], c_sb[d_][:], AF.Tanh)
                        nc.vector.tensor_tensor(h_sb[d_][:], tc_t[:], sigB[:], ALU.mult)
                        if layer == 0:
                            nc.sync.dma_start(
                                out=wfl0[d_, :, :, t, :].transpose([1, 0, 2]),
                                in_=h_sb[d_][:])
                        else:
                            hbf = gp.tile([128, 2, NB], BF16, tag=f"hbf{d_}")
                            nc.vector.tensor_copy(hbf[:], h_sb[d_][:])
                            nc.sync.dma_start(
                                out=wfn[d_, :, :, t, :].transpose([1, 0, 2]),
                                in_=hbf[:])

        # ============ GCN phase 1: degree norms + normalized adjacency ============
        with tc.tile_pool(name="cw", bufs=1) as cw, \
             tc.tile_pool(name="cx", bufs=3) as cx, \
             tc.tile_pool(name="cps", bufs=2, space="PSUM") as cps:
            onescbf_sb = cw.tile([128, 1], BF16, tag="onescbf")
            nc.sync.dma_start(out=onescbf_sb[:], in_=ones_colbf)
            degO = cw.tile([128, NBLK], F32, tag="degO")
            degI = cw.tile([128, NBLK], F32, tag="degI")
            for bl in range(NBLK):
                cbf = cx.tile([128, 128], BF16, tag="cbf")
                nc.sync.dma_start(out=cbf[:], in_=cmat[bl])
                nc.vector.reduce_sum(degO[:, bl:bl + 1], cbf[:],
                                     axis=mybir.AxisListType.X)
                dps = cps.tile([128, 4], F32, tag="dps")
                nc.tensor.matmul(dps[:NS, 0:1], cbf[:NS, 0:NS],
                                 onescbf_sb[:NS, :], start=True, stop=True)
                nc.vector.tensor_copy(degI[:NS, bl:bl + 1], dps[:NS, 0:1])
            rO = cw.tile([128, NBLK], F32, tag="rO")
            rI06 = cw.tile([128, NBLK], F32, tag="rI06")
            nc.vector.tensor_scalar(rO[:], degO[:], 1.0, None, ALU.max)
            nc.vector.reciprocal(rO[:], rO[:])
            nc.scalar.activation(rO[:], rO[:], AF.Sqrt)
            nc.vector.tensor_scalar(rI06[:NS, :], degI[:NS, :], 1.0, None, ALU.max)
            nc.vector.reciprocal(rI06[:NS, :], rI06[:NS, :])
            nc.scalar.activation(rI06[:NS, :], rI06[:NS, :], AF.Sqrt)
            nc.vector.tensor_scalar_mul(rI06[:NS, :], rI06[:NS, :], 0.6)
            nc.gpsimd.memset(rI06g[:], 0.0)
            nc.vector.tensor_copy(rI06g[:NS, :], rI06[:NS, :])
            for bl in range(NBLK):
                cbf2 = cx.tile([128, 128], BF16, tag="cbf2")
                nc.sync.dma_start(out=cbf2[:], in_=cmat[bl])
                msb = cx.tile([128, NS], F32R, tag="msb")
                nc.vector.tensor_scalar(msb[:], cbf2[:, 0:NS], rO[:, bl:bl + 1],
                                        None, ALU.mult)
                nc.sync.dma_start(out=m_dram[bl], in_=msb[:])

        # ============ GCN phase 2 + final ============
        with tc.tile_pool(name="gw", bufs=1) as gw, \
             tc.tile_pool(name="gx", bufs=4) as gx, \
             tc.tile_pool(name="gt", bufs=2) as gt, \
             tc.tile_pool(name="gfin", bufs=1) as gfin, \
             tc.tile_pool(name="gps", bufs=4, space="PSUM") as gps_pool, \
             tc.tile_pool(name="mps", bufs=2, space="PSUM") as mps_pool:

            w1_sb = gw.tile([128, 4, G], BF16, tag="w1g")
            nc.sync.dma_start(out=w1_sb[:], in_=w1g.transpose([1, 0, 2]))
            lin_sb = gw.tile([128, 4, G], BF16, tag="linT")
            nc.sync.dma_start(out=lin_sb[:], in_=linT.transpose([1, 0, 2]))
            linb_sb = gw.tile([1, G], BF16, tag="linb")
            nc.sync.dma_start(out=linb_sb[:], in_=linb)
            w2_sb = gw.tile([128, 2, G], F32R, tag="w2g")
            nc.sync.dma_start(out=w2_sb[:], in_=w2g.transpose([1, 0, 2]))
            ident_sb = gw.tile([128, 128], F32, tag="ident")
            nc.sync.dma_start(out=ident_sb[:], in_=ident)
            onesb_sb = gw.tile([128, 3], F32, tag="onesb")
            nc.sync.dma_start(out=onesb_sb[:], in_=ones_bcol)
            onesrowbf_sb = gw.tile([1, 128], BF16, tag="onesrowbf")
            nc.sync.dma_start(out=onesrowbf_sb[:], in_=ones_rowbf)

            # resident bf16 BiLSTM output [p, kt, t, b]
            wf_sb = gw.tile([128, 4, T, NB], BF16, tag="wf_sb")
            for kt in range(4):
                nc.sync.dma_start(out=wf_sb[:, kt, :, :], in_=wfn[kt // 2, kt % 2])

            rep = gw.tile([128, 2, NB3], F32, tag="rep")

            for bl in range(NBLK):
                # lhsT slot view: [p, r, t] -> slot = T*r + t
                nrep = REM if bl == NBLK - 1 else 3
                nsl = T * nrep
                wrb = gx.tile([128, 4, NS], BF16, tag="wrb")
                nc.vector.tensor_copy(
                    wrb[:, :, 0:nsl].rearrange("p k (r t) -> p k r t", t=T),
                    wf_sb[:, :, :, 3 * bl:3 * bl + nrep].transpose([0, 1, 3, 2]))
                mtl = gx.tile([128, NS], F32R, tag="mtl")
                nc.sync.dma_start(out=mtl[:], in_=m_dram[bl])
                # l2g4 = 0.4*relu(wf@linT + lin_b)
                lps = gps_pool.tile([128, G], F32, tag="gcnps")
                for k in range(4):
                    nc.tensor.matmul(lps[:nsl, :], wrb[:, k, 0:nsl], lin_sb[:, k, :],
                                     start=(k == 0), stop=False)
                nc.tensor.matmul(lps[:nsl, :], onesrowbf_sb[:, 0:nsl], linb_sb[:],
                                 start=False, stop=True)
                l2g4 = gt.tile([128, G], F32, tag="l2g4")
                nc.scalar.activation(l2g4[:nsl, :], lps[:nsl, :], AF.Relu, scale=0.4)
                # p1 = wf@w1
                pps = gps_pool.tile([128, G], F32, tag="gcnps")
                for k in range(4):
                    nc.tensor.matmul(pps[:nsl, :], wrb[:, k, 0:nsl], w1_sb[:, k, :],
                                     start=(k == 0), stop=(k == 3))
                p1 = gt.tile([128, G], F32R, tag="p1")
                nc.vector.tensor_copy(p1[:nsl, :], pps[:nsl, :])
                # agg1 ; h1 = 0.6*relu(rI*agg1) + l2g4
                aps = gps_pool.tile([128, G], F32, tag="gcnps")
                nc.tensor.matmul(aps[:nsl, :], mtl[0:nsl, 0:nsl], p1[:nsl, :],
                                 start=True, stop=True)
                h1 = gt.tile([128, G], F32, tag="h1")
                nc.scalar.activation(h1[:nsl, :], aps[:nsl, :], AF.Relu,
                                     scale=rI06g[:nsl, bl:bl + 1])
                nc.gpsimd.tensor_tensor(h1[:nsl, :], h1[:nsl, :], l2g4[:nsl, :], ALU.add)
                # h1^T via PE transpose
                h1f = gt.tile([128, 2, 128], F32R, tag="h1f")
                for cc_ in range(2):
                    tps = gps_pool.tile([128, G], F32, tag="gcnps")
                    nc.tensor.transpose(tps[:128, 0:nsl],
                                        h1[:nsl, 128 * cc_:128 * (cc_ + 1)],
                                        ident_sb[:nsl, 0:nsl])
                    nc.vector.tensor_copy(h1f[:, cc_, 0:nsl], tps[:128, 0:nsl])
                # p2 ; agg2 ; h2
                p2ps = gps_pool.tile([128, G], F32, tag="gcnps")
                for k in range(2):
                    nc.tensor.matmul(p2ps[:nsl, :], h1f[:, k, 0:nsl], w2_sb[:, k, :],
                                     start=(k == 0), stop=(k == 1))
                p2 = gt.tile([128, G], F32R, tag="p1")
                nc.vector.tensor_copy(p2[:nsl, :], p2ps[:nsl, :])
                a2ps = gps_pool.tile([128, G], F32, tag="gcnps")
                nc.tensor.matmul(a2ps[:nsl, :], mtl[0:nsl, 0:nsl], p2[:nsl, :],
                                 start=True, stop=True)
                h2 = gt.tile([128, G], F32, tag="h2")
                nc.scalar.activation(h2[:nsl, :], a2ps[:nsl, :], AF.Relu,
                                     scale=rI06g[:nsl, bl:bl + 1])
                nc.vector.tensor_tensor(h2[:nsl, :], h2[:nsl, :], l2g4[:nsl, :], ALU.add)
                # rep^T chunk
                for half in range(2):
                    mps = mps_pool.tile([128, 4], F32, tag="meanps")
                    nc.tensor.matmul(mps[:, 0:nrep],
                                     h2[:nsl, 128 * half:128 * (half + 1)],
                                     onesb_sb[:nsl, 0:nrep], start=True, stop=True)
                    nc.vector.tensor_copy(rep[:, half, 3 * bl:3 * bl + nrep],
                                          mps[:, 0:nrep])

            # --- exchange + final head ---
            nc.sync.dma_start(out=cc_in.rearrange("(j p) b -> p j b", p=128),
                              in_=rep[:, :, 0:NB])
            nc.gpsimd.collective_compute(
                "AllGather", ALU.bypass,
                replica_groups=[[0, 4], [1, 5], [2, 6], [3, 7]],
                ins=[cc_in], outs=[cc_out])
            r1 = gfin.tile([128, 2, NB], F32, tag="r1")
            nc.sync.dma_start(out=r1[:], in_=cc_out[0].rearrange("(j p) b -> p j b", p=128))
            r2 = gfin.tile([128, 2, NB], F32, tag="r2")
            nc.sync.dma_start(out=r2[:], in_=cc_out[1].rearrange("(j p) b -> p j b", p=128))
            dist = gfin.tile([128, 2, NB], F32R, tag="dist")
            nc.vector.tensor_tensor(dist[:], r1[:], r2[:], ALU.subtract)
            nc.scalar.activation(dist[:], dist[:], AF.Tanh)
            predT_sb = gw.tile([128, 2, OUT], F32R, tag="predT")
            nc.sync.dma_start(out=predT_sb[:], in_=predT.transpose([1, 0, 2]))
            predb_sb = gw.tile([128, OUT], F32, tag="predb")
            nc.sync.dma_start(out=predb_sb[:], in_=predb)
            for mt in range(ceil_div(NB, 128)):
                mm = min(128, NB - 128 * mt)
                ops = mps_pool.tile([128, 4], F32, tag="meanps")
                for k in range(2):
                    nc.tensor.matmul(ops[:mm, 0:OUT],
                                     dist[:, k, 128 * mt:128 * mt + mm],
                                     predT_sb[:, k, :], start=(k == 0), stop=(k == 1))
                osb = gfin.tile([128, OUT], F32, tag="osb")
                nc.vector.tensor_tensor(osb[:mm, :], ops[:mm, 0:OUT],
                                        predb_sb[:mm, :], ALU.add)
                nc.sync.dma_start(out=out_d[128 * mt:128 * mt + mm, :],
                                  in_=osb[:mm, :])
    return nc


# ====================== host side ======================

def _lstm_weight_pack(p, l, suf):
    """Pack one (layer, dir) weight set into the kernel's tiled layouts."""
    wih = np.asarray(p['w_ih_l%d%s' % (l, suf)], dtype=np.float32)[GPERM]
    whh = np.asarray(p['w_hh_l%d%s' % (l, suf)], dtype=np.float32)[GPERM]
    bias = (np.asarray(p['b_ih_l%d%s' % (l, suf)], dtype=np.float32)
            + np.asarray(p['b_hh_l%d%s' % (l, suf)], dtype=np.float32))[GPERM]
    din = wih.shape[1]
    nkt = 3 if l == 0 else 4
    wi = np.zeros((nkt, 128, 8, 128), dtype=np.float32)
    wiT = wih.T  # [Din, 1024]
    for k in range(nkt):
        rows = wiT[128 * k:128 * (k + 1)]
        wi[k, :rows.shape[0]] = rows.reshape(rows.shape[0], 8, 128)
    if l == 0:
        wi[2, din - 256] = bias.reshape(8, 128)   # bias row at K-row `din`
        bt = None
    else:
        bt = np.ascontiguousarray(bias.reshape(8, 128).T)  # [128, 8]
    wh = np.ascontiguousarray(whh.T.reshape(2, 128, 8, 128))
    return wi, wh, bt


def _build_cmat(src, dst, nb, t):
    """Per-3-graph-block [128,128] edge count matrices; slot = t + (g%3)*T."""
    nblk = ceil_div(nb, 3)
    g = src // t
    s_loc = src % t + (g % 3) * t
    d_loc = dst % t + (g % 3) * t
    bl = g // 3
    flat = bl.astype(np.int64) * (128 * 128) + s_loc.astype(np.int64) * 128 + d_loc
    c = np.bincount(flat, minlength=nblk * 128 * 128).reshape(nblk, 128, 128)
    return c.astype(ml_dtypes.bfloat16)


def _prep_core(x, srcl, dstl, lstm, w1b, w2b, lin_w, lin_b, pred_w, pred_b, nb, t):
    """Build the input map for one core (one branch, nb graphs)."""
    xa = np.zeros((3 * 128, t, nb), dtype=np.float32)
    xa[:D] = np.asarray(x, dtype=np.float32).reshape(nb, t, D).transpose(2, 1, 0)
    xa[D] = 1.0
    m = {"xT": np.ascontiguousarray(xa.reshape(3, 128, t, nb))}
    for d_, suf in ((0, ''), (1, '_r')):
        wi0, wh0, _ = _lstm_weight_pack(lstm, 0, suf)
        wi1, wh1, bt1 = _lstm_weight_pack(lstm, 1, suf)
        m[f"w0i_d{d_}"] = wi0
        m[f"w0h_d{d_}"] = wh0
        m[f"w1i_d{d_}"] = wi1
        m[f"w1h_d{d_}"] = wh1
        m[f"b1_d{d_}"] = bt1
    m["cmat"] = _build_cmat(srcl, dstl, nb, t)
    m["w1g"] = np.asarray(w1b, dtype=np.float32).reshape(4, 128, G).astype(ml_dtypes.bfloat16)
    m["w2g"] = np.ascontiguousarray(
        np.asarray(w2b, dtype=np.float32).reshape(2, 128, G))
    m["linT"] = np.ascontiguousarray(
        np.asarray(lin_w, dtype=np.float32).T.reshape(4, 128, G)).astype(ml_dtypes.bfloat16)
    m["linb"] = np.asarray(lin_b, dtype=np.float32).reshape(1, G).astype(ml_dtypes.bfloat16)
    m["predT"] = np.ascontiguousarray(
        np.asarray(pred_w, dtype=np.float32).T.reshape(2, 128, OUT))
    m["predb"] = np.tile(np.asarray(pred_b, dtype=np.float32).reshape(1, OUT),
                         (128, 1))
    return m


_PROG_CACHE = {}


def _get_program(nb, t):
    key = (nb, t)
    if key not in _PROG_CACHE:
        nc = build_program(nb, t)
        nc.compile()
        _PROG_CACHE[key] = nc
    return _PROG_CACHE[key]


def run(inputs, bg, t):
    """Run the 8-core kernel for a batch of `bg` graphs with `t` nodes each."""
    nb = bg // 4               # graphs per core (4 quarters x 2 branches)
    nbt = nb * t
    nc = _get_program(nb, t)
    in_maps = [dict() for _ in range(NCORES)]
    shared = {
        "linb": np.asarray(inputs['lin_b'], dtype=np.float32).reshape(1, G)
                  .astype(ml_dtypes.bfloat16),
        "linT": np.ascontiguousarray(
            np.asarray(inputs['lin_w'], dtype=np.float32).T.reshape(4, 128, G))
                  .astype(ml_dtypes.bfloat16),
        "predT": np.ascontiguousarray(
            np.asarray(inputs['pred_w'], dtype=np.float32).T.reshape(2, 128, OUT)),
        "predb": np.tile(np.asarray(inputs['pred_b'], dtype=np.float32)
                         .reshape(1, OUT), (128, 1)),
    }
    for br in range(2):
        # one pass per branch, shared by its 4 cores
        x = np.asarray(inputs['x%d' % (br + 1)], dtype=np.float32)
        xa = np.zeros((3 * 128, t, 4 * nb), dtype=np.float32)
        xa[:D] = x.reshape(4 * nb, t, D).transpose(2, 1, 0)
        xa[D] = 1.0
        xa = xa.reshape(3, 128, t, 4 * nb)
        src_f = np.asarray(inputs['src%d' % (br + 1)]).astype(np.int64)
        dst_f = np.asarray(inputs['dst%d' % (br + 1)]).astype(np.int64)
        lstm = inputs['lstm%d' % (br + 1)]
        wm = {}
        for d_, suf in ((0, ''), (1, '_r')):
            wi0, wh0, _ = _lstm_weight_pack(lstm, 0, suf)
            wi1, wh1, bt1 = _lstm_weight_pack(lstm, 1, suf)
            wm.update({f"w0i_d{d_}": wi0, f"w0h_d{d_}": wh0,
                       f"w1i_d{d_}": wi1, f"w1h_d{d_}": wh1, f"b1_d{d_}": bt1})
        wm["w1g"] = np.asarray(inputs['w1%d' % (br + 1)], dtype=np.float32) \
            .reshape(4, 128, G).astype(ml_dtypes.bfloat16)
        wm["w2g"] = np.ascontiguousarray(
            np.asarray(inputs['w2%d' % (br + 1)], dtype=np.float32).reshape(2, 128, G))
        for q in range(4):
            core = br * 4 + q
            esl = slice(q * nb * EPER, (q + 1) * nb * EPER)
            m = in_maps[core]
            m.update(shared)
            m.update(wm)
            m["xT"] = np.ascontiguousarray(xa[:, :, :, q * nb:(q + 1) * nb])
            m["cmat"] = _build_cmat(src_f[esl] - q * nbt, dst_f[esl] - q * nbt,
                                    nb, t)
    res = run_bass_kernel_spmd(nc, in_maps, list(range(NCORES)))
    out = np.empty((bg, OUT), dtype=np.float32)
    for q in range(4):
        out[q * nb:(q + 1) * nb] = res.results[q]["out"]
    return out, res


def kernel(**inputs):
    out, _ = run(inputs, BG_FULL, T_FULL)
    return out
